# revision 10
# baseline (speedup 1.0000x reference)
"""Trainium2 Bass kernel for MixtralAttention (S=2048, H=4096, NH=32, NKV=8, D=128).

Sharding: tensor-parallel over heads across 8 cores. Core c owns q-heads
4c..4c+3 and kv-head c. Everything on-chip runs in a "transposed" (feature
-major) layout so all matmul contractions land on the partition dim with no
on-device transposes of activations:

  stage A: qkvT = wqkvT.T @ xT (bf16 operands, fp32 PSUM), RoPE fused into
           the PSUM eviction; v rotated to [j, d] layout via PE transposes
  stage D: per s-chunk/per-head causal attention, scores kept transposed
           [j, i] so softmax denominators come from a ones-matmul; the
           softmax division is deferred past PV (linearity)
  stage E: chunked AllGather (one per s-chunk) overlapping stage D/F
  stage F: o_projT shard = w_oT_shard.T @ oT_full -> output columns shard,
           per s-chunk as its AllGather lands

Attention/o_proj matmuls run as float32r (full PE rate at free-dim>=256).
Host only slices/casts/transposes inputs and concatenates output shards.
"""

import numpy as np

import concourse.bass as bass
import concourse.mybir as mybir
import concourse.tile as tile
from concourse import bacc, bass_utils
from concourse.masks import make_identity

NCORES = 8
S = 2048
H = 4096
NH = 32
NKV = 8
D = 128
HALF = D // 2
GH = NH // NKV          # 4 q heads per core
QR = GH * D             # 512 q rows per core
NRT = (QR + 2 * D) // 128  # 6 row-tiles of the per-core qkv projection
NSC = S // 512          # 4 s-chunks
NHT = H // 128          # 32 contraction tiles
NJT = S // 128          # 16 key tiles
THETA = 1_000_000.0
SCALE = D ** -0.5

F32 = mybir.dt.float32
F32R = mybir.dt.float32r
BF16 = mybir.dt.bfloat16
AF = mybir.ActivationFunctionType
ALU = mybir.AluOpType

_CACHED_NC = None


def _build_nc():
    nc = bacc.Bacc("TRN2", target_bir_lowering=False, debug=False, num_devices=NCORES)

    xT = nc.dram_tensor("xT", [H, S], BF16, kind="ExternalInput").ap()
    wqkvT = nc.dram_tensor("wqkvT", [H, NRT * 128], BF16, kind="ExternalInput").ap()
    woT = nc.dram_tensor("woT", [H, QR], BF16, kind="ExternalInput").ap()
    cos2 = nc.dram_tensor("cos2", [128, S], F32, kind="ExternalInput").ap()
    sin2s = nc.dram_tensor("sin2s", [128, S], F32, kind="ExternalInput").ap()
    mask4 = nc.dram_tensor("mask4", [128, 4 * 512], F32R, kind="ExternalInput").ap()
    out_sT = nc.dram_tensor("out_sT", [QR, S], F32, kind="ExternalOutput").ap()

    xT_v = xT.rearrange("(ho p) s -> p ho s", p=128)       # [128, 32, 2048]
    wqkvT_v = wqkvT.rearrange("(ho p) r -> p ho r", p=128)  # [128, 32, 768]
    woT_v = woT.rearrange("(ho p) r -> p ho r", p=128)      # [128, 32, 512]

    with tile.TileContext(nc) as tc:
        with (
            tc.tile_pool(name="consts", bufs=1) as cp,
            tc.tile_pool(name="qk_roped", bufs=1) as qkp,
            tc.tile_pool(name="dram", bufs=1, space="DRAM") as dram,
        ):
            # ---- small persistent constants ----
            ones_f = cp.tile([128, 1], F32)
            nc.vector.memset(ones_f[:], 1.0)
            ones_r = cp.tile([128, 1], F32R)
            nc.vector.tensor_copy(ones_r[:], ones_f[:])
            ident = cp.tile([128, 128], F32)
            make_identity(nc, ident[:])

            mask4_sb = cp.tile([128, 4 * 512], F32R)

            # persistent roped q/k (feature-major [d, s]) + v in [j, d] tiles
            qh = [qkp.tile([128, S], F32R, name=f"qh{h}") for h in range(GH)]
            kh = qkp.tile([128, S], F32R)
            vv = qkp.tile([128, NJT, 128], F32R)

            # ---- stage A: fused qkv projection + rope + v transpose ----
            with (
                tc.tile_pool(name="ropec", bufs=1) as rcp,
                tc.tile_pool(name="wsb", bufs=1) as wp,
                tc.tile_pool(name="xstrip", bufs=8) as xp,
                tc.tile_pool(name="ropetmp", bufs=3) as rtp,
                tc.tile_pool(name="vstage", bufs=2) as vsp,
                tc.tile_pool(name="psA", bufs=1, space="PSUM") as psA,
            ):
                cos2_sb = rcp.tile([128, S], F32)
                sin2s_sb = rcp.tile([128, S], F32)
                w_sb = wp.tile([128, NHT, NRT * 128], BF16)
                # interleave w and first-chunk x loads so matmuls start early
                xs0 = []
                for ht in range(NHT):
                    nc.sync.dma_start(w_sb[:, ht, :], wqkvT_v[:, ht, :])
                    xs = xp.tile([128, 512], BF16, name="xs", tag="xs")
                    nc.sync.dma_start(xs[:], xT_v[:, ht, 0:512])
                    xs0.append(xs)
                # constants land behind the first prefetches (needed later)
                nc.sync.dma_start(cos2_sb[:], cos2[:])
                nc.sync.dma_start(sin2s_sb[:], sin2s[:])
                nc.sync.dma_start(mask4_sb[:], mask4[:])

                for sc in range(NSC):
                    ssl = slice(sc * 512, (sc + 1) * 512)
                    ps = [
                        psA.tile([128, 512], F32, name=f"psA{rt}", tag=f"psA{rt}")
                        for rt in range(NRT)
                    ]
                    for ht in range(NHT):
                        if sc == 0:
                            xs = xs0[ht]
                        else:
                            xs = xp.tile([128, 512], BF16, name="xs", tag="xs")
                            nc.sync.dma_start(xs[:], xT_v[:, ht, ssl])
                        for rt in range(NRT):
                            nc.tensor.matmul(
                                ps[rt][:],
                                w_sb[:, ht, rt * 128 : (rt + 1) * 128],
                                xs[:],
                                start=(ht == 0),
                                stop=(ht == NHT - 1),
                            )
                    for rt in (5, 0, 1, 2, 3, 4):
                        if rt < GH + 1:  # q heads and k: apply rope on eviction
                            dst = qh[rt] if rt < GH else kh
                            ta = rtp.tile([128, 512], F32, name="ta", tag="ta")
                            nc.vector.tensor_tensor(
                                ta[:], ps[rt][:], cos2_sb[:, ssl], ALU.mult
                            )
                            tb = rtp.tile([128, 512], F32, name="tb", tag="tb")
                            # out rows 0:64 <- x2 * (-sin); rows 64:128 <- x1 * (+sin)
                            nc.vector.tensor_tensor(
                                tb[0:HALF],
                                ps[rt][HALF:128],
                                sin2s_sb[0:HALF, ssl],
                                ALU.mult,
                            )
                            nc.vector.tensor_tensor(
                                tb[HALF:128],
                                ps[rt][0:HALF],
                                sin2s_sb[HALF:128, ssl],
                                ALU.mult,
                            )
                            nc.vector.tensor_tensor(
                                dst[:, ssl], ta[:], tb[:], ALU.add
                            )
                        else:  # v: evict + PE-transpose into [j, d] tiles
                            vs = vsp.tile([128, 512], F32, name="vs", tag="vs")
                            nc.vector.tensor_copy(vs[:], ps[rt][:])
                            for b in range(4):
                                pt = psA.tile(
                                    [128, 128], F32, name="ptv", tag="ptv", bufs=2
                                )
                                nc.tensor.transpose(
                                    pt[:], vs[:, b * 128 : (b + 1) * 128], ident[:]
                                )
                                nc.vector.tensor_copy(vv[:, sc * 4 + b, :], pt[:])

            # ---- attention + chunked AllGather + o_proj ----
            with (
                tc.tile_pool(name="attn", bufs=1) as ap_,
                tc.tile_pool(name="wo", bufs=1) as wop,
                tc.tile_pool(name="probs", bufs=3) as prp,
                tc.tile_pool(name="ntmp", bufs=2) as ntp,
                tc.tile_pool(name="otn", bufs=3) as onp,
                tc.tile_pool(name="ostrip", bufs=6) as osp,
                tc.tile_pool(name="outsb", bufs=2) as outp,
                tc.tile_pool(name="psD", bufs=1, space="PSUM") as psD,
                tc.tile_pool(name="psF", bufs=1, space="PSUM") as psF,
            ):
                wo_sb = wop.tile([128, NHT, QR], BF16)
                for ho in range(NHT):
                    nc.sync.dma_start(wo_sb[:, ho, :], woT_v[:, ho, :])

                ag_in = [
                    dram.tile([QR, 512], BF16, name=f"ag_in{ic}") for ic in range(NSC)
                ]
                ag_out = [
                    dram.tile(
                        [NCORES * QR, 512], BF16, name=f"ag_out{ic}",
                        addr_space="Shared",
                    )
                    for ic in range(NSC)
                ]

                def stage_d(ic):
                    """Causal attention for query chunk ic, all heads.
                    Returns the last PE matmul for explicit ordering."""
                    isl = slice(ic * 512, (ic + 1) * 512)
                    njt = 4 * (ic + 1)
                    last_mm = None
                    for h in range(GH):
                        psO = psD.tile([128, 512], F32, name="psO", tag="psO")
                        psL = psD.tile([1, 512], F32, name="psL", tag="psL")
                        for jt in range(njt):
                            psS = psD.tile(
                                [128, 512], F32, name="psS", tag="psS", bufs=2
                            )
                            nc.tensor.matmul(
                                psS[:],
                                kh[:, jt * 128 : (jt + 1) * 128],
                                qh[h][:, isl],
                                start=True,
                                stop=True,
                            )
                            pr = prp.tile([128, 512], F32R, name="pr", tag="pr")
                            nc.scalar.activation(pr[:], psS[:], AF.Exp, scale=SCALE)
                            dsub = jt - 4 * ic
                            if dsub >= 0:
                                nc.vector.tensor_tensor(
                                    pr[:],
                                    pr[:],
                                    mask4_sb[:, dsub * 512 : (dsub + 1) * 512],
                                    ALU.mult,
                                )
                            nc.tensor.matmul(
                                psO[:],
                                vv[:, jt, :],
                                pr[:],
                                start=(jt == 0),
                                stop=(jt == njt - 1),
                            )
                            last_mm = nc.tensor.matmul(
                                psL[:],
                                ones_r[:],
                                pr[:],
                                start=(jt == 0),
                                stop=(jt == njt - 1),
                            )
                        # fast PSUM eviction; softmax division happens off the
                        # PE critical path (psO/psL freed immediately)
                        oTu = onp.tile([128, 512], F32, name="oTu", tag="oTu")
                        nc.vector.tensor_copy(oTu[:], psO[:])
                        l1 = ntp.tile([1, 512], F32, name="l1", tag="l1")
                        nc.scalar.activation(l1[:], psL[:], AF.Copy)
                        r1 = ntp.tile([1, 512], F32, name="r1", tag="r1")
                        nc.vector.reciprocal(r1[:], l1[:])
                        rb = ntp.tile([128, 512], F32, name="rb", tag="rb")
                        nc.gpsimd.partition_broadcast(rb[:], r1[:])
                        oTn = onp.tile([128, 512], BF16, name="oTn", tag="oTn")
                        nc.vector.tensor_tensor(oTn[:], oTu[:], rb[:], ALU.mult)
                        nc.sync.dma_start(
                            ag_in[ic][h * 128 : (h + 1) * 128, :], oTn[:]
                        )
                    return last_mm

                def stage_e(ic):
                    nc.gpsimd.collective_compute(
                        "AllGather",
                        ALU.bypass,
                        ins=[ag_in[ic].opt()],
                        outs=[ag_out[ic].opt()],
                        replica_groups=[list(range(NCORES))],
                    )

                def stage_f(ic, after=None):
                    """o_proj for s-chunk ic (depends on its AllGather).

                    `after` pins this stage behind a later attention chunk so
                    the static scheduler cannot hoist its PE matmuls (which
                    wait on the AllGather) into earlier attention work."""
                    isl = slice(ic * 512, (ic + 1) * 512)
                    ag_v = ag_out[ic].rearrange("(jo p) s -> p jo s", p=128)
                    psf = [
                        psF.tile([128, 512], F32, name=f"psF{it}", tag=f"psF{it}")
                        for it in range(4)
                    ]
                    for jo in range(NHT):
                        ost = osp.tile([128, 512], BF16, name="ost", tag="ost")
                        dma = nc.sync.dma_start(ost[:], ag_v[:, jo, :])
                        if after is not None and jo < 6:
                            tile.add_dep_helper(
                                dma.ins,
                                after.ins,
                                sync=False,
                                reason="pipeline o_proj behind later attention",
                            )
                        for it in range(4):
                            nc.tensor.matmul(
                                psf[it][:],
                                wo_sb[:, jo, it * 128 : (it + 1) * 128],
                                ost[:],
                                start=(jo == 0),
                                stop=(jo == NHT - 1),
                            )
                    for it in range(4):
                        osb = outp.tile([128, 512], F32, name="osb", tag="osb")
                        nc.vector.tensor_copy(osb[:], psf[it][:])
                        nc.sync.dma_start(
                            out_sT[it * 128 : (it + 1) * 128, isl], osb[:]
                        )

                # software pipeline: F(ic) is pinned behind D(ic+2) so the
                # in-order PE queue never blocks on an unfinished AllGather
                last = {}
                for ic in range(NSC):
                    last[ic] = stage_d(ic)
                    stage_e(ic)
                for ic in range(NSC):
                    stage_f(ic, after=last[min(ic + 2, NSC - 1)])

    nc.compile()
    return nc


def _host_prep(positions, hidden_states, w_qkv, w_o):
    import ml_dtypes

    pos = np.asarray(positions).astype(np.float32)
    inv_freq = 1.0 / (THETA ** (np.arange(HALF, dtype=np.float32) / HALF))
    ang = pos[:, None] * inv_freq[None, :]            # [S, 64]
    cosT = np.cos(ang).T.astype(np.float32)           # [64, S]
    sinT = np.sin(ang).T.astype(np.float32)
    cos2 = np.ascontiguousarray(np.concatenate([cosT, cosT], axis=0))   # [128, S]
    sin2s = np.ascontiguousarray(np.concatenate([-sinT, sinT], axis=0))

    # boundary-block causal masks: mask4[j, dsub*512 + isub*128 + il]
    jl = np.arange(128)[:, None]
    il = np.arange(128)[None, :]
    tri = (il >= jl).astype(np.float32)
    m4 = np.zeros((128, 4, 4, 128), dtype=np.float32)
    for dsub in range(4):
        for isub in range(4):
            if isub > dsub:
                m4[:, dsub, isub, :] = 1.0
            elif isub == dsub:
                m4[:, dsub, isub, :] = tri
    mask4 = np.ascontiguousarray(m4.reshape(128, 4 * 512))

    xT = np.ascontiguousarray(
        np.asarray(hidden_states).astype(np.float32).T.astype(ml_dtypes.bfloat16)
    )
    w_qkv = np.asarray(w_qkv).astype(np.float32)
    w_o = np.asarray(w_o).astype(np.float32)

    in_maps = []
    for c in range(NCORES):
        wq = w_qkv[c * QR : (c + 1) * QR]                       # [512, H]
        wk = w_qkv[NH * D + c * D : NH * D + (c + 1) * D]       # [128, H]
        wv = w_qkv[(NH + NKV) * D + c * D : (NH + NKV) * D + (c + 1) * D]
        wqkvT = np.ascontiguousarray(
            np.concatenate([wq, wk, wv], axis=0).T.astype(ml_dtypes.bfloat16)
        )
        woT = np.ascontiguousarray(
            w_o[c * QR : (c + 1) * QR, :].T.astype(ml_dtypes.bfloat16)
        )
        in_maps.append(
            {
                "xT": xT,
                "wqkvT": wqkvT,
                "woT": woT,
                "cos2": cos2,
                "sin2s": sin2s,
                "mask4": mask4,
            }
        )
    return in_maps


def kernel(positions, hidden_states, w_qkv, w_o, _run_kwargs=None):
    global _CACHED_NC
    if _CACHED_NC is None:
        _CACHED_NC = _build_nc()
    nc = _CACHED_NC
    in_maps = _host_prep(positions, hidden_states, w_qkv, w_o)
    res = bass_utils.run_bass_kernel_spmd(
        nc, in_maps, core_ids=list(range(NCORES)), **(_run_kwargs or {})
    )
    out = np.empty((S, H), dtype=np.float32)
    for c in range(NCORES):
        out[:, c * QR : (c + 1) * QR] = res.results[c]["out_sT"].T
    if _run_kwargs:
        kernel.last_result = res
    return out
